# revision 14
# baseline (speedup 1.0000x reference)
"""MemoryTree oracle loss kernel for 8 Trainium2 NeuronCores.

Strategy
--------
reference() computes, per level l, logits[b,k,n] = q[b,k] @ mem_l[b,n] @ v[b,k] / D
where mem_l is the pairwise-mean tree built from `leafs`. Because the logit is
linear in the memory matrix and each parent is the *mean* of its children,
level-l logits are exactly pairwise means of level-0 logits. So the only heavy
work is the leaf-level bilinear forms

    s0[b,k,j] = sum_{d,e} leafs[b,j,d,e] * q[b,k,d] * v[b,k,e] / D

one streaming pass over `leafs` (memory-bound). The 12-level log-softmax/NLL
epilogue over 8x4x4096 floats is negligible and done on host in float64.

Device mapping (per core = one batch b)
---------------------------------------
Host pre-transposes leafs[b] to ltT[de, j] (de = 64*64 = 4096 contraction, j =
4096 leaves) so the kernel is a plain GEMM  s0[m, j] = sum_de qv[m, de] *
ltT[de, j]  with a tiny stationary qv and all of ltT streamed through the PE
as the moving operand:

  - 32 row-chunks of 128 (the contraction), 8 PSUM banks = 8 j-groups of 512.
  - data tiles = whole row-chunks over all j -> every DMA is a fully
    contiguous DRAM block (~2MB), max burst efficiency.
  - stationary columns m: for f32r, the 4 query vectors qv[k]/D; for
    bf16/fp8, hi+lo split (qv = hi + lo, both in the low dtype) so stationary
    quantization error is ~squared away; host adds the two psum halves.
  - fp8 uses MatmulPerfMode.DoubleRow: K=256 per matmul (2 chunks), fp8 pairs
    packed per PE cell, 2x matmul throughput.

dtype configs (env KERNEL_CFG): 'f32r' (exact-ish), 'bf16', 'fp8' (default).

Measured (8 cores, interleaved DCE-proof repeat-slope, see measure_hw_time):
  fp8:  ~47 us/pass = 16.8 MB / ~358 GB/s per-NC HBM -> AT the memory
        roofline (PE with DoubleRow ~14-27 us, fully hidden).
        end-to-end loss rel err 1.34e-4 (deterministic; gate is 2e-2) —
        the 12-level log-softmax/CE epilogue attenuates the 3.2e-2 s0
        quantization error by ~200x because the loss is dominated by
        log(num_classes) terms and zero-mean logit noise cancels.
  baseline (previous session's f32 kernel): 327 us. Speedup ~7x ~ headroom.
fp8 e4m3 is the smallest PE-ingestible dtype, so 16.8 MB/core is the traffic
floor: this kernel is at the achievable roofline.
"""

import os
import sys

import numpy as np

# concourse ships on PYTHONPATH in this environment; add known locations as a
# fallback so kernel.py works from a bare directory.
for _p in ("/root/.axon_site/_ro/trn_rl_repo", "/opt/trn_rl_repo"):
    if _p not in sys.path and os.path.isdir(_p):
        sys.path.append(_p)

B = 8
L_K = 4
D = 64
L = 4096
DE = D * D            # contraction length 4096
NJ = 512              # j columns per PSUM bank
NJG = L // NJ         # 8 banks
G = DE // 128         # 32 row-chunks of 128


class Cfg:
    def __init__(self, key: str):
        self.key = key                    # 'f32r' | 'bf16' | 'fp8'
        if key == "f32r":
            self.gpt = 1                  # row-chunks per data tile (2MB)
            self.m = L_K                  # stationary columns
        elif key == "bf16":
            self.gpt = 2
            self.m = 2 * L_K              # hi+lo
        elif key == "fp8":
            self.gpt = 4                  # 2 DoubleRow pairs per tile (2MB)
            self.m = 2 * L_K
        else:
            raise ValueError(key)
        assert G % self.gpt == 0
        self.ntile = G // self.gpt

    @property
    def np_dt(self):
        if self.key == "f32r":
            return np.float32
        import ml_dtypes
        return {"bf16": ml_dtypes.bfloat16,
                "fp8": ml_dtypes.float8_e4m3fn}[self.key]


CFGS = {k: Cfg(k) for k in ("f32r", "bf16", "fp8")}
DEFAULT_CFG = CFGS[os.environ.get("KERNEL_CFG", "fp8")]

TRACE = False
LAST_EXEC_NS = None
LAST_MEAN_EXEC_NS = None
LAST_PROFILE = None

_PROGRAMS = {}


NQ = int(os.environ.get("KERNEL_NQ", "1"))


def _build_program(cfg: Cfg, repeat: int = 1, mode: str = "full",
                   nq: int | None = None):
    nq = NQ if nq is None else nq
    import concourse.bass as bass
    import concourse.tile as tile
    from concourse import bacc, mybir

    f32 = mybir.dt.float32
    ddt = {"f32r": mybir.dt.float32r, "bf16": mybir.dt.bfloat16,
           "fp8": mybir.dt.float8e4}[cfg.key]
    M, GPT, NT = cfg.m, cfg.gpt, cfg.ntile
    dr = cfg.key == "fp8"

    nc = bacc.Bacc(None, target_bir_lowering=False, debug=False)
    lt = nc.declare_dram_parameter("lt", [DE, L], ddt, isOutput=False)
    wt = nc.declare_dram_parameter("wt", [128, G * M], ddt, isOutput=False)
    out = nc.declare_dram_parameter("out", [M, L], f32, isOutput=True)

    with tile.TileContext(nc) as tc:
        with (
            tc.tile_pool(name="consts", bufs=1) as consts,
            tc.tile_pool(name="data", bufs=4) as data_pool,
            tc.tile_pool(name="outp", bufs=1) as outp,
            tc.tile_pool(name="psum", bufs=1, space="PSUM") as psum_pool,
        ):
            wtile = consts.tile([128, G * M], ddt)
            nc.sync.dma_start(out=wtile[:, :], in_=wt[:, :])
            out_sb = outp.tile([M, L], f32)
            if mode in ("acc", "mm", "dma"):
                nc.any.memset(out_sb[:, :], 0.0)

            ps = [
                psum_pool.tile([M, NJ], f32, name=f"ps{i}", tag=f"ps{i}")
                for i in range(NJG)
            ]

            ltv = lt[:, :]

            def tile_ap(t):
                # rows [t*GPT*128, (t+1)*GPT*128) of ltT; fully contiguous.
                return bass.AP(
                    tensor=ltv.tensor,
                    offset=t * GPT * 128 * L,
                    ap=[[L, 128], [128 * L, GPT], [1, L]],
                )

            fixed_dtile = None
            if mode == "mm":
                fixed_dtile = consts.tile([128, GPT * L], ddt)
                nc.sync.dma_start(out=fixed_dtile[:, :], in_=tile_ap(0))

            nbyte = {.0: 0, 1.0: 4}  # unused; keep linters quiet
            del nbyte

            for rep in range(repeat):
                for t in range(NT):
                    if mode == "mm":
                        dt_ = fixed_dtile
                    else:
                        dt_ = data_pool.tile([128, GPT * L], ddt)
                        qeng = [nc.sync, nc.scalar, nc.vector, nc.gpsimd][
                            t % nq]
                        qeng.dma_start(out=dt_[:, :], in_=tile_ap(t))
                    if mode == "dma":
                        # accumulate one word of each tile so every pass's
                        # DMAs stay live (no dead-code elimination)
                        ne = 4 // mybir.dt.size(ddt)
                        sl = out_sb[0:1, t:t + 1]
                        nc.vector.tensor_add(
                            out=sl, in0=sl, in1=dt_[0:1, 0:ne].bitcast(f32),
                        )
                        continue
                    if dr:
                        # pairs of row-chunks, K=256 per matmul
                        dv = dt_.rearrange("p (c i j) -> p c i j", i=2, j=L)
                        wv = wtile.rearrange("p (i c m) -> p i c m", i=2, m=M)
                        for cl in range(GPT // 2):
                            c = t * (GPT // 2) + cl
                            for jg in range(NJG):
                                nc.tensor.matmul(
                                    out=ps[jg][:, :],
                                    lhsT=wv[:, :, c, :],
                                    rhs=dv[:, cl, :, jg * NJ:(jg + 1) * NJ],
                                    start=(c == 0),
                                    stop=(c == G // 2 - 1),
                                    perf_mode=mybir.MatmulPerfMode.DoubleRow,
                                )
                    else:
                        for gl in range(GPT):
                            g = t * GPT + gl
                            for jg in range(NJG):
                                nc.tensor.matmul(
                                    out=ps[jg][:, :],
                                    lhsT=wtile[:, g * M:(g + 1) * M],
                                    rhs=dt_[:, gl * L + jg * NJ:
                                            gl * L + (jg + 1) * NJ],
                                    start=(g == 0),
                                    stop=(g == G - 1),
                                )
                if mode in ("acc", "mm"):
                    for jg in range(NJG):
                        sl = out_sb[:, jg * NJ:(jg + 1) * NJ]
                        nc.vector.tensor_add(out=sl, in0=sl, in1=ps[jg][:, :])
                elif mode != "dma":
                    for jg in range(NJG):
                        if jg % 2 == 0:
                            nc.vector.tensor_copy(
                                out=out_sb[:, jg * NJ:(jg + 1) * NJ],
                                in_=ps[jg][:, :],
                            )
                        else:
                            nc.scalar.copy(
                                out=out_sb[:, jg * NJ:(jg + 1) * NJ],
                                in_=ps[jg][:, :],
                            )

            nc.sync.dma_start(out=out[:, :], in_=out_sb[:, :])

    nc.compile()
    return nc


def _get_program(cfg: Cfg):
    key = cfg.key
    if key not in _PROGRAMS:
        _PROGRAMS[key] = _build_program(cfg)
    return _PROGRAMS[key]


def _build_wmat(cfg: Cfg, qb: np.ndarray, vb: np.ndarray) -> np.ndarray:
    """Stationary for one batch: (128, G*M) in cfg dtype.

    f32r: vecs[k] = (q[k] (x) v[k]) / D, layout wt[p, g*M+m] = vecs[m][g*128+p].
    bf16/fp8: vecs = [hi_0..hi_3, lo_0..lo_3] of qvs = q (x) v (no /D; host
    divides at the end), hi = dt(qvs), lo = dt(qvs - hi).
    fp8 layout (DoubleRow): wt[p, i*(G//2)*M + c*M + m] = vecs[m][(2c+i)*128+p].
    """
    qvs = (qb[:, :, None].astype(np.float64)
           * vb[:, None, :].astype(np.float64)).reshape(L_K, DE)
    if cfg.key == "f32r":
        vecs = (qvs / D).astype(np.float32)
    else:
        dt = cfg.np_dt
        hi = qvs.astype(dt)
        lo = (qvs - hi.astype(np.float64)).astype(dt)
        vecs = np.concatenate([hi, lo], axis=0)          # (M, DE) in dt
    M = cfg.m
    vv = np.asarray(vecs).reshape(M, G, 128)             # [m, g, p]
    if cfg.key == "fp8":
        t = vv.reshape(M, G // 2, 2, 128)                # [m, c, i, p]
        w = t.transpose(3, 2, 1, 0).reshape(128, G * M)  # [p, (i, c, m)]
    else:
        w = vv.transpose(2, 1, 0).reshape(128, G * M)    # [p, (g, m)]
    return np.ascontiguousarray(w)


def _make_in_maps(cfg: Cfg, leafs, q, v):
    dt = cfg.np_dt
    maps = []
    for b in range(B):
        ltT = np.ascontiguousarray(
            np.asarray(leafs[b], np.float32).reshape(L, DE).T).astype(dt)
        maps.append({"lt": ltT, "wt": _build_wmat(cfg, q[b], v[b])})
    return maps


def _combine(cfg: Cfg, out_core: np.ndarray) -> np.ndarray:
    """(M, L) device output -> (L_K, L) s0 for one batch."""
    if cfg.key == "f32r":
        return out_core
    return (out_core[0:L_K] + out_core[L_K:2 * L_K]) / np.float32(D)


def _device_s0(leafs, q, v, cfg: Cfg | None = None) -> np.ndarray:
    """Run the Bass kernel on 8 cores; return s0 (B, L_K, L) float32."""
    global LAST_EXEC_NS, LAST_MEAN_EXEC_NS, LAST_PROFILE
    from concourse.bass_utils import run_bass_kernel_spmd

    cfg = cfg or DEFAULT_CFG
    nc = _get_program(cfg)
    res = run_bass_kernel_spmd(nc, _make_in_maps(cfg, leafs, q, v),
                               list(range(B)), trace=TRACE)
    LAST_EXEC_NS = res.exec_time_ns
    LAST_MEAN_EXEC_NS = res.mean_exec_time_ns
    LAST_PROFILE = res.profile_json
    return np.stack([_combine(cfg, res.results[b]["out"]) for b in range(B)])


def _epilogue(s0: np.ndarray, expected: np.ndarray) -> np.float32:
    """Host float64 epilogue: levels, weighted CE, summed — mirrors reference()."""
    s = s0.astype(np.float64)                        # (B, L_K, L) level-0 logits
    labels0 = expected.astype(np.int64)              # (B, L_K)
    n_labels = B * L_K
    depth = int(round(np.log2(L)))
    total = 0.0
    for level in range(depth):
        if level > 0:
            s = 0.5 * (s[..., 0::2] + s[..., 1::2])
        n_cls = L >> level
        labels = labels0 >> level
        counts = np.bincount(labels.reshape(-1), minlength=n_cls).astype(np.float64)
        w = n_labels / (counts + 1e-8)
        w = w / w.sum()
        mx = s.max(axis=-1, keepdims=True)
        logz = np.log(np.exp(s - mx).sum(axis=-1, keepdims=True)) + mx
        logp_y = np.take_along_axis(s - logz, labels[..., None], axis=-1)[..., 0]
        nll = -logp_y                                # (B, L_K)
        wy = w[labels]
        total += ((wy * nll).sum(axis=0) / wy.sum(axis=0)).sum()
    return np.float32(total)


def kernel(q: np.ndarray, v: np.ndarray, expected: np.ndarray,
           leafs: np.ndarray) -> np.ndarray:
    q = np.asarray(q, dtype=np.float32)
    v = np.asarray(v, dtype=np.float32)
    expected = np.asarray(expected)
    leafs = np.asarray(leafs, dtype=np.float32)
    assert q.shape == (B, L_K, D) and leafs.shape == (B, L, D, D)
    s0 = _device_s0(leafs, q, v)
    return np.asarray(_epilogue(s0, expected))


def benchmark(q, v, leafs, iters: int = 20, repeat: int = 1,
              mode: str = "full", cfg: Cfg | None = None):
    """Time the sharded PJRT executable with device-resident inputs.

    Returns (per_call_seconds_list, pipelined_avg_seconds, s0) where s0 is the
    combined result from the last call (for sanity checking).
    """
    import time

    import jax
    import numpy as np_
    from jax.sharding import Mesh, NamedSharding, PartitionSpec
    try:
        from jax.experimental.shard_map import shard_map
    except ImportError:
        from jax.shard_map import shard_map
    from concourse import bass2jax, mybir

    cfg = cfg or DEFAULT_CFG
    bass2jax.install_neuronx_cc_hook()
    nc = (_get_program(cfg) if repeat == 1 and mode == "full"
          else _build_program(cfg, repeat, mode))

    partition_name = (nc.partition_id_tensor.name
                      if nc.partition_id_tensor else None)
    in_names, out_names, out_avals, zero_shapes = [], [], [], []
    for alloc in nc.m.functions[0].allocations:
        if not isinstance(alloc, mybir.MemoryLocationSet):
            continue
        name = alloc.memorylocations[0].name
        if alloc.kind == "ExternalInput":
            if name != partition_name:
                in_names.append(name)
        elif alloc.kind == "ExternalOutput":
            out_names.append(name)
            shape = tuple(alloc.tensor_shape)
            dtype = mybir.dt.np(alloc.dtype)
            out_avals.append(jax.core.ShapedArray(shape, dtype))
            zero_shapes.append((shape, dtype))
    n_params = len(in_names)
    n_outs = len(out_avals)
    all_names = in_names + out_names
    if partition_name is not None:
        all_names = all_names + [partition_name]

    def _body(*args):
        operands = list(args)
        if partition_name is not None:
            operands.append(bass2jax.partition_id_tensor())
        outs = bass2jax._bass_exec_p.bind(
            *operands,
            out_avals=tuple(out_avals),
            in_names=tuple(all_names),
            out_names=tuple(out_names),
            lowering_input_output_aliases=(),
            sim_require_finite=True,
            sim_require_nnan=True,
            nc=nc,
        )
        return tuple(outs)

    devices = jax.devices()[:B]
    mesh = Mesh(np_.asarray(devices), ("core",))
    donate = tuple(range(n_params, n_params + n_outs))
    sharded = jax.jit(
        shard_map(
            _body, mesh=mesh,
            in_specs=(PartitionSpec("core"),) * (n_params + n_outs),
            out_specs=(PartitionSpec("core"),) * n_outs,
            check_rep=False,
        ),
        donate_argnums=donate, keep_unused=True,
    )

    in_maps = _make_in_maps(cfg, leafs, q, v)
    concat_in = [
        np_.concatenate([in_maps[c][nm] for c in range(B)], axis=0)
        for nm in in_names
    ]
    concat_in_dev = [
        jax.device_put(a, NamedSharding(mesh, PartitionSpec("core")))
        for a in concat_in
    ]

    def zeros():
        return [np_.zeros((B * s[0], *s[1:]), d) for s, d in zero_shapes]

    # warmup (includes compile)
    out = sharded(*concat_in_dev, *zeros())
    jax.block_until_ready(out)

    times = []
    last = None
    for _ in range(iters):
        t0 = time.perf_counter()
        out = sharded(*concat_in_dev, *zeros())
        jax.block_until_ready(out)
        times.append(time.perf_counter() - t0)
        last = out

    # pipelined: dispatch all, block once
    t0 = time.perf_counter()
    outs = [sharded(*concat_in_dev, *zeros()) for _ in range(iters)]
    jax.block_until_ready(outs)
    pipelined = (time.perf_counter() - t0) / iters

    oidx = out_names.index("out")
    full = np_.asarray(last[oidx]).reshape(B, cfg.m, L)
    s0 = np_.stack([_combine(cfg, full[b]) for b in range(B)])
    return times, pipelined, s0


def measure_hw_time(q, v, leafs, cfg: Cfg | None = None, mode: str = "acc",
                    r_lo: int = 32, r_hi: int = 544, rounds: int = 12):
    """Robust device-time-per-pass measurement.

    Builds two programs whose PSUM drains ACCUMULATE across `repeat` passes
    (out = R * s0 — every pass is live, so no pass can be dead-code
    eliminated; the output scale proves execution). Times them with
    interleaved blocking calls so axon dispatch-overhead drift cancels, and
    returns the paired-median slope (seconds per pass) plus diagnostics.
    """
    import time as _time

    import jax
    import numpy as np_
    from jax.sharding import Mesh, NamedSharding, PartitionSpec
    try:
        from jax.experimental.shard_map import shard_map
    except ImportError:
        from jax.shard_map import shard_map
    from concourse import bass2jax, mybir

    cfg = cfg or DEFAULT_CFG
    bass2jax.install_neuronx_cc_hook()
    in_maps = _make_in_maps(cfg, leafs, q, v)

    def make_runner(repeat):
        nc = _build_program(cfg, repeat, mode)
        partition_name = (nc.partition_id_tensor.name
                          if nc.partition_id_tensor else None)
        in_names, out_names, out_avals, zero_shapes = [], [], [], []
        for alloc in nc.m.functions[0].allocations:
            if not isinstance(alloc, mybir.MemoryLocationSet):
                continue
            name = alloc.memorylocations[0].name
            if alloc.kind == "ExternalInput":
                if name != partition_name:
                    in_names.append(name)
            elif alloc.kind == "ExternalOutput":
                out_names.append(name)
                shape = tuple(alloc.tensor_shape)
                dtype = mybir.dt.np(alloc.dtype)
                out_avals.append(jax.core.ShapedArray(shape, dtype))
                zero_shapes.append((shape, dtype))
        n_params = len(in_names)
        all_names = in_names + out_names
        if partition_name is not None:
            all_names = all_names + [partition_name]

        def _body(*args):
            operands = list(args)
            if partition_name is not None:
                operands.append(bass2jax.partition_id_tensor())
            return tuple(bass2jax._bass_exec_p.bind(
                *operands, out_avals=tuple(out_avals),
                in_names=tuple(all_names), out_names=tuple(out_names),
                lowering_input_output_aliases=(),
                sim_require_finite=True, sim_require_nnan=True, nc=nc))

        devices = jax.devices()[:B]
        mesh = Mesh(np_.asarray(devices), ("core",))
        donate = tuple(range(n_params, n_params + len(out_avals)))
        sharded = jax.jit(
            shard_map(_body, mesh=mesh,
                      in_specs=(PartitionSpec("core"),) * (n_params
                                                           + len(out_avals)),
                      out_specs=(PartitionSpec("core"),) * len(out_avals),
                      check_rep=False),
            donate_argnums=donate, keep_unused=True)
        concat_in = [
            np_.concatenate([in_maps[c][nm] for c in range(B)], axis=0)
            for nm in in_names
        ]
        dev_in = [jax.device_put(a, NamedSharding(mesh, PartitionSpec("core")))
                  for a in concat_in]

        def zeros():
            return [np_.zeros((B * s[0], *s[1:]), d) for s, d in zero_shapes]

        state = {}

        def run_once():
            t0 = _time.perf_counter()
            out = sharded(*dev_in, *zeros())
            jax.block_until_ready(out)
            state["out"] = out[out_names.index("out")]
            return _time.perf_counter() - t0

        for _ in range(2):  # compile + warm
            run_once()
        return run_once, state

    run_lo, st_lo = make_runner(r_lo)
    run_hi, st_hi = make_runner(r_hi)
    lo, hi = [], []
    for _ in range(rounds):
        lo.append(run_lo())
        hi.append(run_hi())
    lo, hi = np.array(lo), np.array(hi)
    slope = float(np.median((hi - lo) / (r_hi - r_lo)))
    scale = float(np.linalg.norm(np.asarray(st_hi["out"], np.float64))
                  / max(np.linalg.norm(np.asarray(st_lo["out"], np.float64)),
                        1e-30))
    return {
        "t_pass": slope,
        "med_lo_ms": float(np.median(lo) * 1e3),
        "med_hi_ms": float(np.median(hi) * 1e3),
        "r_lo": r_lo, "r_hi": r_hi,
        "exec_scale": scale, "expected_scale": r_hi / r_lo,
    }


def _selftest_numpy():
    """Validate index math (wmat layout + combine) in pure numpy."""
    rng = np.random.default_rng(0)
    q = rng.standard_normal((B, L_K, D)).astype(np.float32)
    v = rng.standard_normal((B, L_K, D)).astype(np.float32)
    leafs = rng.standard_normal((1, L, D, D)).astype(np.float32)
    b = 0
    ref = np.einsum('kd,jde,ke->kj', q[b].astype(np.float64),
                    leafs[b].astype(np.float64),
                    v[b].astype(np.float64)) / D
    for cfg in CFGS.values():
        M = cfg.m
        wm = _build_wmat(cfg, q[b], v[b]).astype(np.float64)  # (128, G*M)
        ltT = np.ascontiguousarray(
            leafs[b].reshape(L, DE).T).astype(cfg.np_dt).astype(np.float64)
        out = np.zeros((M, L), np.float64)
        if cfg.key == "fp8":
            wv = wm.reshape(128, 2, G // 2, M)              # p, i, c, m
            for c in range(G // 2):
                for i in range(2):
                    rows = ltT[(2 * c + i) * 128:(2 * c + i + 1) * 128]
                    out += wv[:, i, c, :].T @ rows
        else:
            wv = wm.reshape(128, G, M)
            for g in range(G):
                out += wv[:, g, :].T @ ltT[g * 128:(g + 1) * 128]
        s0 = _combine(cfg, out.astype(np.float32))
        err = np.abs(s0 - ref).max() / np.abs(ref).max()
        print(f"{cfg.key}: selftest rel err {err:.2e}")
        lim = {"f32r": 1e-5, "bf16": 2e-2, "fp8": 2e-1}[cfg.key]
        assert err < lim, (cfg.key, err)
    print("selftest OK")


if __name__ == "__main__":
    _selftest_numpy()
